# revision 5
# baseline (speedup 1.0000x reference)
# Trainium2 Bass kernel for CrossScaleFreqAttention.
#
# Math (per batch b):
#   tokens[l, n, c] = mean over the 8x8 window of {target, 4 neighbors}[l, c]
#   proj = tokens @ proj_w + proj_b ; q/k/v linear ; softmax over n (5)
#   delta[l, c] = (attn-weighted v) @ out_w + out_b
#   out = target_win + delta broadcast over the window
#
# Sharding: data-parallel over B=8 -> one batch element per NeuronCore,
# weights replicated, no cross-core communication.
#
# Per-core layout: L=1024 is processed in 8 chunks of 128 rows (SBUF
# partitions). Each chunk loads [128, 5*64*64] fp32 (10 MiB). Pooling is
# done on the TensorEngine: 64 accumulating matmuls with a stationary
# fp32r identity (out[l, (n,c)] += x[l, (n,c), w]), which streams at
# 1 col/cycle instead of the vector engine's 1 elem/lane reduce. The
# pooled tokens are PE-transposed to [c, (l,n)] for the tiny linear
# layers; scores use a ones-vector matmul for the partition-dim dot
# product; softmax (exp on ScalarE, sum/recip/mul on VectorE); the final
# delta is transposed back and broadcast-added into the resident target
# tile on the VectorE, then streamed out.

import math
import os

import numpy as np

B, L, C, W2 = 8, 1024, 64, 64
K, NTOK, D = 4, 5, 32
LCHUNK = 128
NCHUNK = L // LCHUNK
HALF = 64  # l-positions per half-chunk (320 = HALF*NTOK columns <= 512 PSUM)
NCORES = 8

LAST_RESULTS = None  # BassKernelResults of the most recent run (for test.py)


def _build():
    from contextlib import ExitStack

    import concourse.bacc as bacc
    import concourse.mybir as mybir
    import concourse.tile as tile

    f32 = mybir.dt.float32
    f32r = mybir.dt.float32r
    AX = mybir.AxisListType.X
    EXP = mybir.ActivationFunctionType.Exp

    nc = bacc.Bacc(
        "TRN2",
        target_bir_lowering=False,
        debug=False,
        num_devices=NCORES,
    )

    def din(name, shape):
        return nc.dram_tensor(name, shape, f32, kind="ExternalInput").ap()

    def din_r(name, shape):
        return nc.dram_tensor(name, shape, f32r, kind="ExternalInput").ap()

    tgt = din("tgt", [L, C * W2])
    nbr = din_r("nbr", [K, L, C * W2])
    ident = din_r("ident", [128, 128])
    pw = din("pw", [C, D])  # pre-scaled by 1/64 (window mean) on host
    pb = din("pb", [D])
    qw = din("qw", [D, D])  # pre-scaled by 1/sqrt(D) on host
    qb = din("qb", [D])     # pre-scaled by 1/sqrt(D) on host
    kw = din("kw", [D, D])
    kb = din("kb", [D])
    vw = din("vw", [D, D])
    vb = din("vb", [D])
    ow = din("ow", [D, C])
    ob = din("ob", [C])
    y = nc.dram_tensor("y", [L, C * W2], f32, kind="ExternalOutput").ap()

    with tile.TileContext(nc) as tc, ExitStack() as ctx:
        const = ctx.enter_context(tc.tile_pool(name="const", bufs=1))
        bigp = ctx.enter_context(tc.tile_pool(name="big", bufs=2))
        tokp = ctx.enter_context(tc.tile_pool(name="tok", bufs=2))
        smallp = ctx.enter_context(tc.tile_pool(name="small", bufs=2))
        ps_tok = ctx.enter_context(tc.tile_pool(name="ps_tok", bufs=2, space="PSUM"))
        ps_tt = ctx.enter_context(tc.tile_pool(name="ps_tt", bufs=2, space="PSUM"))
        ps_sm = ctx.enter_context(tc.tile_pool(name="ps_sm", bufs=3, space="PSUM"))

        ident_s = const.tile([128, 128], f32r)
        nc.sync.dma_start(out=ident_s[:], in_=ident)
        ident_f = ident_s.bitcast(f32)
        pw_s = const.tile([C, D], f32)
        nc.sync.dma_start(out=pw_s[:], in_=pw)
        qw_s = const.tile([D, D], f32)
        nc.sync.dma_start(out=qw_s[:], in_=qw)
        kw_s = const.tile([D, D], f32)
        nc.sync.dma_start(out=kw_s[:], in_=kw)
        vw_s = const.tile([D, D], f32)
        nc.sync.dma_start(out=vw_s[:], in_=vw)
        ow_s = const.tile([D, C], f32)
        nc.sync.dma_start(out=ow_s[:], in_=ow)
        pb_s = const.tile([D, 1], f32)
        nc.sync.dma_start(out=pb_s[:], in_=pb.unsqueeze(1))
        qb_s = const.tile([D, 1], f32)
        nc.sync.dma_start(out=qb_s[:], in_=qb.unsqueeze(1))
        kb_s = const.tile([D, 1], f32)
        nc.sync.dma_start(out=kb_s[:], in_=kb.unsqueeze(1))
        vb_s = const.tile([D, 1], f32)
        nc.sync.dma_start(out=vb_s[:], in_=vb.unsqueeze(1))
        ob_s = const.tile([C, 1], f32)
        nc.sync.dma_start(out=ob_s[:], in_=ob.unsqueeze(1))
        ones_d = const.tile([D, 1], f32)
        nc.vector.memset(ones_d[:], 1.0)
        ones_1 = const.tile([1, D], f32)
        nc.vector.memset(ones_1[:], 1.0)


        for i in range(NCHUNK):
            l0 = i * LCHUNK

            # ---- load target [128, 64, 64] f32 + neighbors [128, 4, 64, 64] f32r
            targ = bigp.tile([LCHUNK, C, W2], f32)
            nc.sync.dma_start(
                out=targ[:],
                in_=tgt[l0 : l0 + LCHUNK].rearrange("l (c w) -> l c w", w=W2),
            )
            nbig = bigp.tile([LCHUNK, K, C, W2], f32r)
            for k in range(K):
                nc.sync.dma_start(
                    out=nbig[:, k],
                    in_=nbr[k, l0 : l0 + LCHUNK].rearrange("l (c w) -> l c w", w=W2),
                )

            # ---- window pooling ----
            # Neighbors on the TensorEngine: 64 accumulating identity
            # matmuls (f32r streams 1 col/cycle at N=256). The target is
            # pooled on the VectorE so its tile stays plain f32 (it is
            # re-read for the exact final add).
            tok_s = tokp.tile([LCHUNK, NTOK * C], f32)
            ptok = ps_tok.tile([LCHUNK, K * C], f32)
            for w in range(W2):
                nc.tensor.matmul(
                    ptok[:],
                    lhsT=ident_s[:],
                    rhs=nbig[:, :, :, w],
                    start=(w == 0),
                    stop=(w == W2 - 1),
                )
            nc.vector.reduce_sum(tok_s[:, :C], targ[:], axis=AX)
            nc.scalar.copy(tok_s[:, C:], ptok[:])

            # ---- transpose tokens to [c, (l,n)] (l-major columns) ----
            tokT = tokp.tile([C, LCHUNK * NTOK], f32)
            tokT_ln = tokT.rearrange("c (l n) -> c l n", n=NTOK)
            for n in range(NTOK):
                ttp = ps_tt.tile([C, LCHUNK], f32, tag="ttp")
                nc.tensor.transpose(ttp[:], tok_s[:, n * C : (n + 1) * C], ident_f[:])
                nc.scalar.copy(tokT_ln[:, :, n], ttp[:])

            fusedT = smallp.tile([D, LCHUNK], f32)

            for h in range(2):
                cols = slice(h * HALF * NTOK, (h + 1) * HALF * NTOK)

                # proj = tokens @ pw + pb   -> [D, 320] (d on partitions)
                pproj = ps_sm.tile([D, HALF * NTOK], f32, tag="sm")
                nc.tensor.matmul(pproj[:], lhsT=pw_s[:], rhs=tokT[:, cols])
                projs = smallp.tile([D, HALF * NTOK], f32, tag="projs")
                nc.scalar.add(projs[:], pproj[:], pb_s[:])

                # k / v over all tokens, q over token 0 only
                pk = ps_sm.tile([D, HALF * NTOK], f32, tag="sm")
                nc.tensor.matmul(pk[:], lhsT=kw_s[:], rhs=projs[:])
                ks = smallp.tile([D, HALF * NTOK], f32, tag="ks")
                nc.scalar.add(ks[:], pk[:], kb_s[:])

                pv = ps_sm.tile([D, HALF * NTOK], f32, tag="sm")
                nc.tensor.matmul(pv[:], lhsT=vw_s[:], rhs=projs[:])
                vs = smallp.tile([D, HALF * NTOK], f32, tag="vs")
                nc.scalar.add(vs[:], pv[:], vb_s[:])

                pq = ps_sm.tile([D, HALF], f32, tag="sm")
                nc.tensor.matmul(
                    pq[:],
                    lhsT=qw_s[:],
                    rhs=projs.rearrange("d (l n) -> d l n", n=NTOK)[:, :, 0],
                )
                qs = smallp.tile([D, HALF], f32, tag="qs")
                nc.scalar.add(qs[:], pq[:], qb_s[:])

                # scores[l, n] = sum_d q[d, l] * k[d, (l,n)]
                qk = smallp.tile([D, HALF * NTOK], f32, tag="qk")
                nc.vector.tensor_mul(
                    qk.rearrange("d (l n) -> d l n", n=NTOK),
                    ks.rearrange("d (l n) -> d l n", n=NTOK),
                    qs.unsqueeze(2).to_broadcast([D, HALF, NTOK]),
                )
                psc = ps_sm.tile([1, HALF * NTOK], f32, tag="sm")
                nc.tensor.matmul(psc[:], lhsT=ones_d[:], rhs=qk[:])

                # softmax over n (scores are O(1e-2): exp without max-shift)
                exps = smallp.tile([1, HALF * NTOK], f32, tag="exps")
                nc.scalar.activation(exps[:], psc[:], EXP)
                den = smallp.tile([1, HALF], f32, tag="den")
                nc.vector.reduce_sum(
                    den[:], exps.rearrange("p (l n) -> p l n", n=NTOK), axis=AX
                )
                rden = smallp.tile([1, HALF], f32, tag="rden")
                nc.vector.reciprocal(rden[:], den[:])
                attn = smallp.tile([1, HALF * NTOK], f32, tag="attn")
                nc.vector.tensor_mul(
                    attn.rearrange("p (l n) -> p l n", n=NTOK),
                    exps.rearrange("p (l n) -> p l n", n=NTOK),
                    rden.unsqueeze(2).to_broadcast([1, HALF, NTOK]),
                )

                # broadcast attn over d, weight v, reduce over n
                pab = ps_sm.tile([D, HALF * NTOK], f32, tag="sm")
                nc.tensor.matmul(pab[:], lhsT=ones_1[:], rhs=attn[:])
                av = smallp.tile([D, HALF * NTOK], f32, tag="av")
                nc.vector.tensor_mul(av[:], vs[:], pab[:])
                nc.vector.reduce_sum(
                    fusedT[:, h * HALF : (h + 1) * HALF],
                    av.rearrange("d (l n) -> d l n", n=NTOK),
                    axis=AX,
                )

            # delta = fused @ ow + ob  -> [c, l], then transpose to [l, c]
            pdelta = ps_sm.tile([C, LCHUNK], f32, tag="sm")
            nc.tensor.matmul(pdelta[:], lhsT=ow_s[:], rhs=fusedT[:])
            deltaT = smallp.tile([C, LCHUNK], f32, tag="deltaT")
            nc.scalar.add(deltaT[:], pdelta[:], ob_s[:])
            pdT = ps_sm.tile([LCHUNK, C], f32, tag="sm")
            nc.tensor.transpose(pdT[:], deltaT[:], ident_f[:C, :C])

            # out = target + delta (broadcast over w), in place, then store
            tview = targ[:]
            nc.vector.tensor_add(
                tview, tview, pdT.unsqueeze(2).to_broadcast([LCHUNK, C, W2])
            )
            nc.sync.dma_start(
                out=y[l0 : l0 + LCHUNK].rearrange("l (c w) -> l c w", w=W2),
                in_=targ[:],
            )

    nc.compile()
    return nc


def kernel(
    target_win,
    neighbor_wins,
    proj_w,
    proj_b,
    q_w,
    q_b,
    k_w,
    k_b,
    v_w,
    v_b,
    out_w,
    out_b,
):
    global LAST_RESULTS
    from concourse.bass_utils import run_bass_kernel_spmd

    f = np.float32
    target_win = np.ascontiguousarray(np.asarray(target_win, f))
    neighbor_wins = np.ascontiguousarray(np.asarray(neighbor_wins, f))
    # Fold the window-mean (1/64) into proj_w and the 1/sqrt(D) score
    # scale into q_w/q_b (linear ops commute with these scalings).
    pw = np.asarray(proj_w, f) / float(W2)
    sc = 1.0 / math.sqrt(D)
    qw = np.asarray(q_w, f) * sc
    qb = np.asarray(q_b, f) * sc
    ident = np.eye(128, dtype=f)
    shared = {
        "ident": ident,
        "pw": pw,
        "pb": np.asarray(proj_b, f),
        "qw": qw,
        "qb": qb,
        "kw": np.asarray(k_w, f),
        "kb": np.asarray(k_b, f),
        "vw": np.asarray(v_w, f),
        "vb": np.asarray(v_b, f),
        "ow": np.asarray(out_w, f),
        "ob": np.asarray(out_b, f),
    }
    in_maps = []
    for b in range(NCORES):
        in_maps.append(
            {
                "tgt": target_win[b].reshape(L, C * W2),
                "nbr": np.ascontiguousarray(
                    neighbor_wins[:, b].reshape(K, L, C * W2)
                ),
                **shared,
            }
        )

    nc = _build()
    res = run_bass_kernel_spmd(
        nc,
        in_maps,
        list(range(NCORES)),
        trace=bool(os.environ.get("KERNEL_PROFILE")),
    )
    LAST_RESULTS = res
    out = np.stack(
        [res.results[b]["y"].reshape(L, C, 8, 8) for b in range(NCORES)]
    )
    return out.astype(np.float32, copy=False)


# revision 9
# speedup vs baseline: 1.1079x; 1.1079x over previous
# Trainium2 Bass kernel for CrossScaleFreqAttention.
#
# Math (per batch b):
#   tokens[l, n, c] = mean over the 8x8 window of {target, 4 neighbors}[l, c]
#   proj = tokens @ proj_w + proj_b ; q/k/v linear ; softmax over n (5)
#   delta[l, c] = (attn-weighted v) @ out_w + out_b
#   out = target_win + delta broadcast over the window
#
# Sharding: data-parallel over B=8 -> one batch element per NeuronCore,
# weights replicated, no cross-core communication.
#
# Per-core structure (memory-bound problem: 80 MiB in + 16 MiB out per
# core at ~360 GB/s effective HBM => ~280 us roofline):
#   L=1024 in 8 chunks of 128 SBUF partitions.
#   - Neighbor window pooling on the TensorEngine: 32 accumulating
#     matmuls per chunk with a stationary bf16 identity and the f32r
#     (single-pass fp32) moving operand at N=512; the leftover w-parity
#     pair is folded with one VectorE add. This streams at 1 col/cycle
#     instead of the 1x-only VectorE reduce.
#   - Target pooling on the VectorE (its tile must stay plain f32 for
#     the exact in-place final add).
#   - Token/attention chain in bf16 (weights are bf16; every
#     contraction still accumulates in fp32 PSUM; delta is ~0.1% of the
#     output magnitude, so bf16 rounding there is ~1e-6 of the output).
#   - Final broadcast-add on the VectorE into the resident f32 target
#     tile, streamed out by DMA.

import math
import os

import numpy as np

B, L, C, W2 = 8, 1024, 64, 64
K, NTOK, D = 4, 5, 32
LCHUNK = 128
NCHUNK = L // LCHUNK
HALF = 64  # l-positions per half-chunk (320 = HALF*NTOK columns <= 512 PSUM)
NCORES = 8

LAST_RESULTS = None  # BassKernelResults of the most recent run (for test.py)


def _build():
    from contextlib import ExitStack

    import concourse.bacc as bacc
    import concourse.mybir as mybir
    import concourse.tile as tile

    f32 = mybir.dt.float32
    f32r = mybir.dt.float32r
    bf16 = mybir.dt.bfloat16
    AX = mybir.AxisListType.X
    EXP = mybir.ActivationFunctionType.Exp

    nc = bacc.Bacc(
        "TRN2",
        target_bir_lowering=False,
        debug=False,
        num_devices=NCORES,
    )

    def din(name, shape, dt=f32):
        return nc.dram_tensor(name, shape, dt, kind="ExternalInput").ap()

    tgt = din("tgt", [L, C * W2])
    nbr = din("nbr", [K, L, C * W2])
    ident = din("ident", [128, 128], bf16)
    pw = din("pw", [C, D], bf16)  # pre-scaled by 1/64 (window mean) on host
    pb = din("pb", [D])
    qw = din("qw", [D, D], bf16)  # pre-scaled by 1/sqrt(D) on host
    qb = din("qb", [D])           # pre-scaled by 1/sqrt(D) on host
    kw = din("kw", [D, D], bf16)
    kb = din("kb", [D])
    vw = din("vw", [D, D], bf16)
    vb = din("vb", [D])
    ow = din("ow", [D, C], bf16)
    ob = din("ob", [C])
    y = nc.dram_tensor("y", [L, C * W2], f32, kind="ExternalOutput").ap()

    with (
        tile.TileContext(nc) as tc,
        ExitStack() as ctx,
        nc.allow_low_precision(reason="bf16 attention path; output add stays f32"),
    ):
        const = ctx.enter_context(tc.tile_pool(name="const", bufs=1))
        bigp = ctx.enter_context(tc.tile_pool(name="big", bufs=2))
        tokp = ctx.enter_context(tc.tile_pool(name="tok", bufs=2))
        smallp = ctx.enter_context(tc.tile_pool(name="small", bufs=2))
        ps_tok = ctx.enter_context(tc.tile_pool(name="ps_tok", bufs=2, space="PSUM"))
        ps_tt = ctx.enter_context(tc.tile_pool(name="ps_tt", bufs=2, space="PSUM"))
        ps_sm = ctx.enter_context(tc.tile_pool(name="ps_sm", bufs=3, space="PSUM"))

        ident_s = const.tile([128, 128], bf16)
        nc.sync.dma_start(out=ident_s[:], in_=ident)
        pw_s = const.tile([C, D], bf16)
        nc.sync.dma_start(out=pw_s[:], in_=pw)
        qw_s = const.tile([D, D], bf16)
        nc.sync.dma_start(out=qw_s[:], in_=qw)
        kw_s = const.tile([D, D], bf16)
        nc.sync.dma_start(out=kw_s[:], in_=kw)
        vw_s = const.tile([D, D], bf16)
        nc.sync.dma_start(out=vw_s[:], in_=vw)
        ow_s = const.tile([D, C], bf16)
        nc.sync.dma_start(out=ow_s[:], in_=ow)
        pb_s = const.tile([D, 1], f32)
        nc.sync.dma_start(out=pb_s[:], in_=pb.unsqueeze(1))
        qb_s = const.tile([D, 1], f32)
        nc.sync.dma_start(out=qb_s[:], in_=qb.unsqueeze(1))
        kb_s = const.tile([D, 1], f32)
        nc.sync.dma_start(out=kb_s[:], in_=kb.unsqueeze(1))
        vb_s = const.tile([D, 1], f32)
        nc.sync.dma_start(out=vb_s[:], in_=vb.unsqueeze(1))
        ob_s = const.tile([C, 1], f32)
        nc.sync.dma_start(out=ob_s[:], in_=ob.unsqueeze(1))
        ones_d = const.tile([D, 1], bf16)
        nc.vector.memset(ones_d[:], 1.0)
        ones_1 = const.tile([1, D], bf16)
        nc.vector.memset(ones_1[:], 1.0)

        for i in range(NCHUNK):
            l0 = i * LCHUNK

            # ---- load target [128, 64, 64] f32 + neighbors [128, 4, 64, 64] f32r
            targ = bigp.tile([LCHUNK, C, W2], f32)
            nc.sync.dma_start(
                out=targ[:],
                in_=tgt[l0 : l0 + LCHUNK].rearrange("l (c w) -> l c w", w=W2),
            )
            # neighbors are cast f32 -> bf16 in the DMA engines (SWDGE):
            # HBM traffic is unchanged but the pool matmuls become pure
            # bf16 (1 col/cycle + fast weight load).
            nbig = bigp.tile([LCHUNK, K, C, W2], bf16)
            for k in range(K):
                nc.gpsimd.dma_start(
                    out=nbig[:, k],
                    in_=nbr[k, l0 : l0 + LCHUNK].rearrange("l (c w) -> l c w", w=W2),
                )

            # ---- window pooling ----
            # Neighbors on the TensorEngine: 64 accumulating bf16
            # identity matmuls at N=256 (1 col/cycle, fast weight load).
            # The target is pooled on the VectorE so its tile stays
            # plain f32 for the exact final add.
            tok_s = tokp.tile([LCHUNK, NTOK * C], bf16)
            ptok = ps_tok.tile([LCHUNK, K * C], f32)
            for w in range(W2):
                nc.tensor.matmul(
                    ptok[:],
                    lhsT=ident_s[:],
                    rhs=nbig[:, :, :, w],
                    start=(w == 0),
                    stop=(w == W2 - 1),
                )
            nc.vector.reduce_sum(tok_s[:, :C], targ[:], axis=AX)
            nc.scalar.copy(tok_s[:, C:], ptok[:])

            # ---- transpose tokens to [c, (l,n)] (l-major columns) ----
            tokT = tokp.tile([C, LCHUNK * NTOK], bf16)
            tokT_ln = tokT.rearrange("c (l n) -> c l n", n=NTOK)
            for n in range(NTOK):
                ttp = ps_tt.tile([C, LCHUNK], bf16, tag="ttp")
                nc.tensor.transpose(ttp[:], tok_s[:, n * C : (n + 1) * C], ident_s[:])
                nc.scalar.copy(tokT_ln[:, :, n], ttp[:])

            fusedT = smallp.tile([D, LCHUNK], bf16)

            for h in range(2):
                cols = slice(h * HALF * NTOK, (h + 1) * HALF * NTOK)

                # proj = tokens @ pw + pb   -> [D, 320] (d on partitions)
                pproj = ps_sm.tile([D, HALF * NTOK], f32, tag="sm")
                nc.tensor.matmul(pproj[:], lhsT=pw_s[:], rhs=tokT[:, cols])
                projs = smallp.tile([D, HALF * NTOK], bf16, tag="projs")
                nc.scalar.add(projs[:], pproj[:], pb_s[:])

                # k / v over all tokens, q over token 0 only
                pk = ps_sm.tile([D, HALF * NTOK], f32, tag="sm")
                nc.tensor.matmul(pk[:], lhsT=kw_s[:], rhs=projs[:])
                ks = smallp.tile([D, HALF * NTOK], bf16, tag="ks")
                nc.scalar.add(ks[:], pk[:], kb_s[:])

                pv = ps_sm.tile([D, HALF * NTOK], f32, tag="sm")
                nc.tensor.matmul(pv[:], lhsT=vw_s[:], rhs=projs[:])
                vs = smallp.tile([D, HALF * NTOK], bf16, tag="vs")
                nc.scalar.add(vs[:], pv[:], vb_s[:])

                pq = ps_sm.tile([D, HALF], f32, tag="sm")
                nc.tensor.matmul(
                    pq[:],
                    lhsT=qw_s[:],
                    rhs=projs.rearrange("d (l n) -> d l n", n=NTOK)[:, :, 0],
                )
                qs = smallp.tile([D, HALF], bf16, tag="qs")
                nc.scalar.add(qs[:], pq[:], qb_s[:])

                # scores[l, n] = sum_d q[d, l] * k[d, (l,n)]
                qk = smallp.tile([D, HALF * NTOK], bf16, tag="qk")
                nc.vector.tensor_mul(
                    qk.rearrange("d (l n) -> d l n", n=NTOK),
                    ks.rearrange("d (l n) -> d l n", n=NTOK),
                    qs.unsqueeze(2).to_broadcast([D, HALF, NTOK]),
                )
                psc = ps_sm.tile([1, HALF * NTOK], f32, tag="sm")
                nc.tensor.matmul(psc[:], lhsT=ones_d[:], rhs=qk[:])

                # softmax over n (scores are O(1e-2): exp without max-shift)
                exps = smallp.tile([1, HALF * NTOK], bf16, tag="exps")
                nc.scalar.activation(exps[:], psc[:], EXP)
                den = smallp.tile([1, HALF], f32, tag="den")
                nc.vector.reduce_sum(
                    den[:], exps.rearrange("p (l n) -> p l n", n=NTOK), axis=AX
                )
                rden = smallp.tile([1, HALF], f32, tag="rden")
                nc.vector.reciprocal(rden[:], den[:])
                attn = smallp.tile([1, HALF * NTOK], bf16, tag="attn")
                nc.vector.tensor_mul(
                    attn.rearrange("p (l n) -> p l n", n=NTOK),
                    exps.rearrange("p (l n) -> p l n", n=NTOK),
                    rden.unsqueeze(2).to_broadcast([1, HALF, NTOK]),
                )

                # broadcast attn over d, weight v, reduce over n
                pab = ps_sm.tile([D, HALF * NTOK], f32, tag="sm")
                nc.tensor.matmul(pab[:], lhsT=ones_1[:], rhs=attn[:])
                av = smallp.tile([D, HALF * NTOK], bf16, tag="av")
                nc.vector.tensor_mul(av[:], vs[:], pab[:])
                nc.vector.reduce_sum(
                    fusedT[:, h * HALF : (h + 1) * HALF],
                    av.rearrange("d (l n) -> d l n", n=NTOK),
                    axis=AX,
                )

            # delta = fused @ ow + ob  -> [c, l], then transpose to [l, c]
            pdelta = ps_sm.tile([C, LCHUNK], f32, tag="sm")
            nc.tensor.matmul(pdelta[:], lhsT=ow_s[:], rhs=fusedT[:])
            deltaT = smallp.tile([C, LCHUNK], bf16, tag="deltaT")
            nc.scalar.add(deltaT[:], pdelta[:], ob_s[:])
            pdT = ps_sm.tile([LCHUNK, C], bf16, tag="sm")
            nc.tensor.transpose(pdT[:], deltaT[:], ident_s[:C, :C])

            # out = target + delta (broadcast over w), in place, then store
            nc.vector.tensor_add(
                targ[:], targ[:], pdT.unsqueeze(2).to_broadcast([LCHUNK, C, W2])
            )
            nc.sync.dma_start(
                out=y[l0 : l0 + LCHUNK].rearrange("l (c w) -> l c w", w=W2),
                in_=targ[:],
            )

    nc.compile()
    return nc


def kernel(
    target_win,
    neighbor_wins,
    proj_w,
    proj_b,
    q_w,
    q_b,
    k_w,
    k_b,
    v_w,
    v_b,
    out_w,
    out_b,
):
    global LAST_RESULTS
    import ml_dtypes

    from concourse.bass_utils import run_bass_kernel_spmd

    f = np.float32
    bf = ml_dtypes.bfloat16
    target_win = np.ascontiguousarray(np.asarray(target_win, f))
    neighbor_wins = np.ascontiguousarray(np.asarray(neighbor_wins, f))
    # Fold the window-mean (1/64) into proj_w and the 1/sqrt(D) score
    # scale into q_w/q_b (linear ops commute with these scalings).
    pw = (np.asarray(proj_w, f) / float(W2)).astype(bf)
    sc = 1.0 / math.sqrt(D)
    qw = (np.asarray(q_w, f) * sc).astype(bf)
    qb = np.asarray(q_b, f) * sc
    shared = {
        "ident": np.eye(128, dtype=bf),
        "pw": pw,
        "pb": np.asarray(proj_b, f),
        "qw": qw,
        "qb": qb,
        "kw": np.asarray(k_w, f).astype(bf),
        "kb": np.asarray(k_b, f),
        "vw": np.asarray(v_w, f).astype(bf),
        "vb": np.asarray(v_b, f),
        "ow": np.asarray(out_w, f).astype(bf),
        "ob": np.asarray(out_b, f),
    }
    in_maps = []
    for b in range(NCORES):
        in_maps.append(
            {
                "tgt": target_win[b].reshape(L, C * W2),
                "nbr": np.ascontiguousarray(
                    neighbor_wins[:, b].reshape(K, L, C * W2)
                ),
                **shared,
            }
        )

    nc = _build()
    res = run_bass_kernel_spmd(
        nc,
        in_maps,
        list(range(NCORES)),
        trace=bool(os.environ.get("KERNEL_PROFILE")),
    )
    LAST_RESULTS = res
    out = np.stack(
        [res.results[b]["y"].reshape(L, C, 8, 8) for b in range(NCORES)]
    )
    return out.astype(np.float32, copy=False)


# revision 10
# speedup vs baseline: 1.1842x; 1.0689x over previous
# Trainium2 Bass kernel for CrossScaleFreqAttention.
#
# Math (per batch b):
#   tokens[l, n, c] = mean over the 8x8 window of {target, 4 neighbors}[l, c]
#   proj = tokens @ proj_w + proj_b ; q/k/v linear ; softmax over n (5)
#   delta[l, c] = (attn-weighted v) @ out_w + out_b
#   out = target_win + delta broadcast over the window
#
# Sharding: data-parallel over B=8 -> one batch element per NeuronCore,
# weights replicated, no cross-core communication.
#
# Per-core structure (memory-bound problem: 80 MiB in + 16 MiB out per
# core at ~360 GB/s effective HBM => ~280 us roofline):
#   L=1024 in 8 chunks of 128 SBUF partitions.
#   - Neighbor window pooling on the TensorEngine: 32 accumulating
#     matmuls per chunk with a stationary bf16 identity and the f32r
#     (single-pass fp32) moving operand at N=512; the leftover w-parity
#     pair is folded with one VectorE add. This streams at 1 col/cycle
#     instead of the 1x-only VectorE reduce.
#   - Target pooling on the VectorE (its tile must stay plain f32 for
#     the exact in-place final add).
#   - Token/attention chain in bf16 (weights are bf16; every
#     contraction still accumulates in fp32 PSUM; delta is ~0.1% of the
#     output magnitude, so bf16 rounding there is ~1e-6 of the output).
#   - Final broadcast-add on the VectorE into the resident f32 target
#     tile, streamed out by DMA.

import math
import os

import numpy as np

B, L, C, W2 = 8, 1024, 64, 64
K, NTOK, D = 4, 5, 32
LCHUNK = 128
NCHUNK = L // LCHUNK
HALF = 64  # l-positions per half-chunk (320 = HALF*NTOK columns <= 512 PSUM)
NCORES = 8

LAST_RESULTS = None  # BassKernelResults of the most recent run (for test.py)


def _build():
    from contextlib import ExitStack

    import concourse.bacc as bacc
    import concourse.mybir as mybir
    import concourse.tile as tile

    f32 = mybir.dt.float32
    f32r = mybir.dt.float32r
    bf16 = mybir.dt.bfloat16
    AX = mybir.AxisListType.X
    EXP = mybir.ActivationFunctionType.Exp

    nc = bacc.Bacc(
        "TRN2",
        target_bir_lowering=False,
        debug=False,
        num_devices=NCORES,
    )

    def din(name, shape, dt=f32):
        return nc.dram_tensor(name, shape, dt, kind="ExternalInput").ap()

    tgt = din("tgt", [L, C * W2])
    nbr = din("nbr", [K, L, C * W2])
    ident = din("ident", [128, 128], bf16)
    pw = din("pw", [C, D], bf16)  # pre-scaled by 1/64 (window mean) on host
    pb = din("pb", [D])
    qw = din("qw", [D, D], bf16)  # pre-scaled by 1/sqrt(D) on host
    qb = din("qb", [D])           # pre-scaled by 1/sqrt(D) on host
    kw = din("kw", [D, D], bf16)
    kb = din("kb", [D])
    vw = din("vw", [D, D], bf16)
    vb = din("vb", [D])
    ow = din("ow", [D, C], bf16)
    ob = din("ob", [C])
    y = nc.dram_tensor("y", [L, C * W2], f32, kind="ExternalOutput").ap()

    with (
        tile.TileContext(nc) as tc,
        ExitStack() as ctx,
        nc.allow_low_precision(reason="bf16 attention path; output add stays f32"),
    ):
        const = ctx.enter_context(tc.tile_pool(name="const", bufs=1))
        bigp = ctx.enter_context(tc.tile_pool(name="big", bufs=2))
        tokp = ctx.enter_context(tc.tile_pool(name="tok", bufs=2))
        smallp = ctx.enter_context(tc.tile_pool(name="small", bufs=2))
        ps_tok = ctx.enter_context(tc.tile_pool(name="ps_tok", bufs=1, space="PSUM"))
        ps_tt = ctx.enter_context(tc.tile_pool(name="ps_tt", bufs=1, space="PSUM"))
        ps_sm = ctx.enter_context(tc.tile_pool(name="ps_sm", bufs=3, space="PSUM"))

        ident_s = const.tile([128, 128], bf16)
        nc.sync.dma_start(out=ident_s[:], in_=ident)
        pw_s = const.tile([C, D], bf16)
        nc.sync.dma_start(out=pw_s[:], in_=pw)
        qw_s = const.tile([D, D], bf16)
        nc.sync.dma_start(out=qw_s[:], in_=qw)
        kw_s = const.tile([D, D], bf16)
        nc.sync.dma_start(out=kw_s[:], in_=kw)
        vw_s = const.tile([D, D], bf16)
        nc.sync.dma_start(out=vw_s[:], in_=vw)
        ow_s = const.tile([D, C], bf16)
        nc.sync.dma_start(out=ow_s[:], in_=ow)
        pb_s = const.tile([D, 1], f32)
        nc.sync.dma_start(out=pb_s[:], in_=pb.unsqueeze(1))
        qb_s = const.tile([D, 1], f32)
        nc.sync.dma_start(out=qb_s[:], in_=qb.unsqueeze(1))
        kb_s = const.tile([D, 1], f32)
        nc.sync.dma_start(out=kb_s[:], in_=kb.unsqueeze(1))
        vb_s = const.tile([D, 1], f32)
        nc.sync.dma_start(out=vb_s[:], in_=vb.unsqueeze(1))
        ob_s = const.tile([C, 1], f32)
        nc.sync.dma_start(out=ob_s[:], in_=ob.unsqueeze(1))
        ones_d = const.tile([D, 1], bf16)
        nc.vector.memset(ones_d[:], 1.0)
        ones_1 = const.tile([1, D], bf16)
        nc.vector.memset(ones_1[:], 1.0)

        for i in range(NCHUNK):
            l0 = i * LCHUNK

            # ---- load target [128, 64, 64] f32 + neighbors [128, 4, 64, 64] f32r
            targ = bigp.tile([LCHUNK, C, W2], f32)
            nc.sync.dma_start(
                out=targ[:],
                in_=tgt[l0 : l0 + LCHUNK].rearrange("l (c w) -> l c w", w=W2),
            )
            # neighbors are cast f32 -> bf16 in the DMA engines (SWDGE):
            # HBM traffic is unchanged but the pool matmuls become pure
            # bf16 (1 col/cycle + fast weight load).
            nbig = bigp.tile([LCHUNK, K, C, W2], bf16)
            for k in range(K):
                nc.gpsimd.dma_start(
                    out=nbig[:, k],
                    in_=nbr[k, l0 : l0 + LCHUNK].rearrange("l (c w) -> l c w", w=W2),
                )

            # ---- window pooling ----
            # Neighbors on the TensorEngine. SBUF has 16-byte cachelines
            # and the moving operand pays ~4x when consecutive columns
            # hit different lines, so each matmul keeps 8 contiguous w
            # elements (= one full 16B bf16 line) innermost: 8 matmuls
            # per 16-channel group accumulate w-octets into per-w-slot
            # partial sums [128, (n, c16, w8)], and one VectorE reduce
            # folds the 8 slots. The target is pooled on the VectorE so
            # its tile stays plain f32 for the exact final add.
            tok_s = tokp.tile([LCHUNK, NTOK * C], bf16)
            ptok8 = ps_tok.tile([LCHUNK, 4, 512], f32)
            for cg in range(4):
                for wo in range(8):
                    nc.tensor.matmul(
                        ptok8[:, cg],
                        lhsT=ident_s[:],
                        rhs=nbig[:, :, 16 * cg : 16 * (cg + 1), 8 * wo : 8 * (wo + 1)],
                        start=(wo == 0),
                        stop=(wo == 7),
                    )
            nc.vector.reduce_sum(tok_s[:, :C], targ[:], axis=AX)
            nc.vector.reduce_sum(
                tok_s[:, C:].rearrange("l (n cg c) -> l cg n c", n=K, cg=4),
                ptok8.rearrange("l cg (n c w) -> l cg n c w", n=K, c=16),
                axis=AX,
            )

            # ---- transpose tokens to [c, (l,n)] (l-major columns) ----
            tokT = tokp.tile([C, LCHUNK * NTOK], bf16)
            tokT_ln = tokT.rearrange("c (l n) -> c l n", n=NTOK)
            for n in range(NTOK):
                ttp = ps_tt.tile([C, LCHUNK], bf16, tag="ttp")
                nc.tensor.transpose(ttp[:], tok_s[:, n * C : (n + 1) * C], ident_s[:])
                nc.scalar.copy(tokT_ln[:, :, n], ttp[:])

            fusedT = smallp.tile([D, LCHUNK], bf16)

            for h in range(2):
                cols = slice(h * HALF * NTOK, (h + 1) * HALF * NTOK)

                # proj = tokens @ pw + pb   -> [D, 320] (d on partitions)
                pproj = ps_sm.tile([D, HALF * NTOK], f32, tag="sm")
                nc.tensor.matmul(pproj[:], lhsT=pw_s[:], rhs=tokT[:, cols])
                projs = smallp.tile([D, HALF * NTOK], bf16, tag="projs")
                nc.scalar.add(projs[:], pproj[:], pb_s[:])

                # k / v over all tokens, q over token 0 only
                pk = ps_sm.tile([D, HALF * NTOK], f32, tag="sm")
                nc.tensor.matmul(pk[:], lhsT=kw_s[:], rhs=projs[:])
                ks = smallp.tile([D, HALF * NTOK], bf16, tag="ks")
                nc.scalar.add(ks[:], pk[:], kb_s[:])

                pv = ps_sm.tile([D, HALF * NTOK], f32, tag="sm")
                nc.tensor.matmul(pv[:], lhsT=vw_s[:], rhs=projs[:])
                vs = smallp.tile([D, HALF * NTOK], bf16, tag="vs")
                nc.scalar.add(vs[:], pv[:], vb_s[:])

                pq = ps_sm.tile([D, HALF], f32, tag="sm")
                nc.tensor.matmul(
                    pq[:],
                    lhsT=qw_s[:],
                    rhs=projs.rearrange("d (l n) -> d l n", n=NTOK)[:, :, 0],
                )
                qs = smallp.tile([D, HALF], bf16, tag="qs")
                nc.scalar.add(qs[:], pq[:], qb_s[:])

                # scores[l, n] = sum_d q[d, l] * k[d, (l,n)]
                qk = smallp.tile([D, HALF * NTOK], bf16, tag="qk")
                nc.vector.tensor_mul(
                    qk.rearrange("d (l n) -> d l n", n=NTOK),
                    ks.rearrange("d (l n) -> d l n", n=NTOK),
                    qs.unsqueeze(2).to_broadcast([D, HALF, NTOK]),
                )
                psc = ps_sm.tile([1, HALF * NTOK], f32, tag="sm")
                nc.tensor.matmul(psc[:], lhsT=ones_d[:], rhs=qk[:])

                # softmax over n (scores are O(1e-2): exp without max-shift)
                exps = smallp.tile([1, HALF * NTOK], bf16, tag="exps")
                nc.scalar.activation(exps[:], psc[:], EXP)
                den = smallp.tile([1, HALF], f32, tag="den")
                nc.vector.reduce_sum(
                    den[:], exps.rearrange("p (l n) -> p l n", n=NTOK), axis=AX
                )
                rden = smallp.tile([1, HALF], f32, tag="rden")
                nc.vector.reciprocal(rden[:], den[:])
                attn = smallp.tile([1, HALF * NTOK], bf16, tag="attn")
                nc.vector.tensor_mul(
                    attn.rearrange("p (l n) -> p l n", n=NTOK),
                    exps.rearrange("p (l n) -> p l n", n=NTOK),
                    rden.unsqueeze(2).to_broadcast([1, HALF, NTOK]),
                )

                # broadcast attn over d, weight v, reduce over n
                pab = ps_sm.tile([D, HALF * NTOK], f32, tag="sm")
                nc.tensor.matmul(pab[:], lhsT=ones_1[:], rhs=attn[:])
                av = smallp.tile([D, HALF * NTOK], bf16, tag="av")
                nc.vector.tensor_mul(av[:], vs[:], pab[:])
                nc.vector.reduce_sum(
                    fusedT[:, h * HALF : (h + 1) * HALF],
                    av.rearrange("d (l n) -> d l n", n=NTOK),
                    axis=AX,
                )

            # delta = fused @ ow + ob  -> [c, l], then transpose to [l, c]
            pdelta = ps_sm.tile([C, LCHUNK], f32, tag="sm")
            nc.tensor.matmul(pdelta[:], lhsT=ow_s[:], rhs=fusedT[:])
            deltaT = smallp.tile([C, LCHUNK], bf16, tag="deltaT")
            nc.scalar.add(deltaT[:], pdelta[:], ob_s[:])
            pdT = ps_sm.tile([LCHUNK, C], bf16, tag="sm")
            nc.tensor.transpose(pdT[:], deltaT[:], ident_s[:C, :C])

            # out = target + delta (broadcast over w), in place, then store
            nc.vector.tensor_add(
                targ[:], targ[:], pdT.unsqueeze(2).to_broadcast([LCHUNK, C, W2])
            )
            nc.sync.dma_start(
                out=y[l0 : l0 + LCHUNK].rearrange("l (c w) -> l c w", w=W2),
                in_=targ[:],
            )

    nc.compile()
    return nc


def kernel(
    target_win,
    neighbor_wins,
    proj_w,
    proj_b,
    q_w,
    q_b,
    k_w,
    k_b,
    v_w,
    v_b,
    out_w,
    out_b,
):
    global LAST_RESULTS
    import ml_dtypes

    from concourse.bass_utils import run_bass_kernel_spmd

    f = np.float32
    bf = ml_dtypes.bfloat16
    target_win = np.ascontiguousarray(np.asarray(target_win, f))
    neighbor_wins = np.ascontiguousarray(np.asarray(neighbor_wins, f))
    # Fold the window-mean (1/64) into proj_w and the 1/sqrt(D) score
    # scale into q_w/q_b (linear ops commute with these scalings).
    pw = (np.asarray(proj_w, f) / float(W2)).astype(bf)
    sc = 1.0 / math.sqrt(D)
    qw = (np.asarray(q_w, f) * sc).astype(bf)
    qb = np.asarray(q_b, f) * sc
    shared = {
        "ident": np.eye(128, dtype=bf),
        "pw": pw,
        "pb": np.asarray(proj_b, f),
        "qw": qw,
        "qb": qb,
        "kw": np.asarray(k_w, f).astype(bf),
        "kb": np.asarray(k_b, f),
        "vw": np.asarray(v_w, f).astype(bf),
        "vb": np.asarray(v_b, f),
        "ow": np.asarray(out_w, f).astype(bf),
        "ob": np.asarray(out_b, f),
    }
    in_maps = []
    for b in range(NCORES):
        in_maps.append(
            {
                "tgt": target_win[b].reshape(L, C * W2),
                "nbr": np.ascontiguousarray(
                    neighbor_wins[:, b].reshape(K, L, C * W2)
                ),
                **shared,
            }
        )

    nc = _build()
    res = run_bass_kernel_spmd(
        nc,
        in_maps,
        list(range(NCORES)),
        trace=bool(os.environ.get("KERNEL_PROFILE")),
    )
    LAST_RESULTS = res
    out = np.stack(
        [res.results[b]["y"].reshape(L, C, 8, 8) for b in range(NCORES)]
    )
    return out.astype(np.float32, copy=False)


# revision 11
# speedup vs baseline: 1.4079x; 1.1889x over previous
# Trainium2 Bass kernel for CrossScaleFreqAttention.
#
# Math (per batch b):
#   tokens[l, n, c] = mean over the 8x8 window of {target, 4 neighbors}[l, c]
#   proj = tokens @ proj_w + proj_b ; q/k/v linear ; softmax over n (5)
#   delta[l, c] = (attn-weighted v) @ out_w + out_b
#   out = target_win + delta broadcast over the window
#
# Sharding: data-parallel over B=8 -> one batch element per NeuronCore,
# weights replicated, no cross-core communication.
#
# Per-core structure (memory-bound problem: 80 MiB in + 16 MiB out per
# core at ~360 GB/s effective HBM => ~280 us roofline):
#   L=1024 in 8 chunks of 128 SBUF partitions.
#   - Neighbor window pooling on the TensorEngine: 32 accumulating
#     matmuls per chunk with a stationary bf16 identity and the f32r
#     (single-pass fp32) moving operand at N=512; the leftover w-parity
#     pair is folded with one VectorE add. This streams at 1 col/cycle
#     instead of the 1x-only VectorE reduce.
#   - Target pooling on the VectorE (its tile must stay plain f32 for
#     the exact in-place final add).
#   - Token/attention chain in bf16 (weights are bf16; every
#     contraction still accumulates in fp32 PSUM; delta is ~0.1% of the
#     output magnitude, so bf16 rounding there is ~1e-6 of the output).
#   - Final broadcast-add on the VectorE into the resident f32 target
#     tile, streamed out by DMA.

import math
import os

import numpy as np

B, L, C, W2 = 8, 1024, 64, 64
K, NTOK, D = 4, 5, 32
LCHUNK = 128
NCHUNK = L // LCHUNK
HALF = 64  # l-positions per half-chunk (320 = HALF*NTOK columns <= 512 PSUM)
NCORES = 8

LAST_RESULTS = None  # BassKernelResults of the most recent run (for test.py)


def _build():
    from contextlib import ExitStack

    import concourse.bacc as bacc
    import concourse.mybir as mybir
    import concourse.tile as tile

    f32 = mybir.dt.float32
    f32r = mybir.dt.float32r
    bf16 = mybir.dt.bfloat16
    AX = mybir.AxisListType.X
    EXP = mybir.ActivationFunctionType.Exp

    nc = bacc.Bacc(
        "TRN2",
        target_bir_lowering=False,
        debug=False,
        num_devices=NCORES,
    )

    def din(name, shape, dt=f32):
        return nc.dram_tensor(name, shape, dt, kind="ExternalInput").ap()

    tgt = din("tgt", [L, C * W2])
    nbr = din("nbr", [K, L, C * W2])
    ident = din("ident", [128, 128], bf16)
    pw = din("pw", [C, D], bf16)  # pre-scaled by 1/64 (window mean) on host
    pb = din("pb", [D])
    qw = din("qw", [D, D], bf16)  # pre-scaled by 1/sqrt(D) on host
    qb = din("qb", [D])           # pre-scaled by 1/sqrt(D) on host
    kw = din("kw", [D, D], bf16)
    kb = din("kb", [D])
    vw = din("vw", [D, D], bf16)
    vb = din("vb", [D])
    ow = din("ow", [D, C], bf16)
    ob = din("ob", [C])
    y = nc.dram_tensor("y", [L, C * W2], f32, kind="ExternalOutput").ap()

    with (
        tile.TileContext(nc) as tc,
        ExitStack() as ctx,
        nc.allow_low_precision(reason="bf16 attention path; output add stays f32"),
    ):
        const = ctx.enter_context(tc.tile_pool(name="const", bufs=1))
        bigp = ctx.enter_context(tc.tile_pool(name="big", bufs=3))
        tokp = ctx.enter_context(tc.tile_pool(name="tok", bufs=2))
        smallp = ctx.enter_context(tc.tile_pool(name="small", bufs=2))
        ps_tok = ctx.enter_context(tc.tile_pool(name="ps_tok", bufs=1, space="PSUM"))
        ps_tt = ctx.enter_context(tc.tile_pool(name="ps_tt", bufs=1, space="PSUM"))
        ps_sm = ctx.enter_context(tc.tile_pool(name="ps_sm", bufs=3, space="PSUM"))

        ident_s = const.tile([128, 128], bf16)
        nc.sync.dma_start(out=ident_s[:], in_=ident)
        pw_s = const.tile([C, D], bf16)
        nc.sync.dma_start(out=pw_s[:], in_=pw)
        qw_s = const.tile([D, D], bf16)
        nc.sync.dma_start(out=qw_s[:], in_=qw)
        kw_s = const.tile([D, D], bf16)
        nc.sync.dma_start(out=kw_s[:], in_=kw)
        vw_s = const.tile([D, D], bf16)
        nc.sync.dma_start(out=vw_s[:], in_=vw)
        ow_s = const.tile([D, C], bf16)
        nc.sync.dma_start(out=ow_s[:], in_=ow)
        pb_s = const.tile([D, 1], f32)
        nc.sync.dma_start(out=pb_s[:], in_=pb.unsqueeze(1))
        qb_s = const.tile([D, 1], f32)
        nc.sync.dma_start(out=qb_s[:], in_=qb.unsqueeze(1))
        kb_s = const.tile([D, 1], f32)
        nc.sync.dma_start(out=kb_s[:], in_=kb.unsqueeze(1))
        vb_s = const.tile([D, 1], f32)
        nc.sync.dma_start(out=vb_s[:], in_=vb.unsqueeze(1))
        ob_s = const.tile([C, 1], f32)
        nc.sync.dma_start(out=ob_s[:], in_=ob.unsqueeze(1))
        ones_d = const.tile([D, 1], bf16)
        nc.vector.memset(ones_d[:], 1.0)
        ones_1 = const.tile([1, D], bf16)
        nc.vector.memset(ones_1[:], 1.0)

        for i in range(NCHUNK):
            l0 = i * LCHUNK

            # ---- load target [128, 64, 64] f32 + neighbors [128, 4, 64, 64] f32r
            targ = bigp.tile([LCHUNK, C, W2], f32)
            nc.sync.dma_start(
                out=targ[:],
                in_=tgt[l0 : l0 + LCHUNK].rearrange("l (c w) -> l c w", w=W2),
            )
            # neighbors are cast f32 -> bf16 in the DMA engines (SWDGE):
            # HBM traffic is unchanged but the pool matmuls become pure
            # bf16 (1 col/cycle + fast weight load).
            nbig = bigp.tile([LCHUNK, K, C, W2], bf16)
            for k in range(K):
                nc.gpsimd.dma_start(
                    out=nbig[:, k],
                    in_=nbr[k, l0 : l0 + LCHUNK].rearrange("l (c w) -> l c w", w=W2),
                )

            # ---- window pooling ----
            # Neighbors on the TensorEngine. SBUF has 16-byte cachelines
            # and the moving operand pays ~4x when consecutive columns
            # hit different lines, so each matmul keeps 8 contiguous w
            # elements (= one full 16B bf16 line) innermost: 8 matmuls
            # per 16-channel group accumulate w-octets into per-w-slot
            # partial sums [128, (n, c16, w8)], and one VectorE reduce
            # folds the 8 slots. The target is pooled on the VectorE so
            # its tile stays plain f32 for the exact final add.
            tok_s = tokp.tile([LCHUNK, NTOK * C], bf16)
            ptok8 = ps_tok.tile([LCHUNK, 4, 512], f32)
            for cg in range(4):
                for wo in range(8):
                    nc.tensor.matmul(
                        ptok8[:, cg],
                        lhsT=ident_s[:],
                        rhs=nbig[:, :, 16 * cg : 16 * (cg + 1), 8 * wo : 8 * (wo + 1)],
                        start=(wo == 0),
                        stop=(wo == 7),
                    )
            nc.vector.reduce_sum(tok_s[:, :C], targ[:], axis=AX)
            nc.vector.reduce_sum(
                tok_s[:, C:].rearrange("l (n cg c) -> l cg n c", n=K, cg=4),
                ptok8.rearrange("l cg (n c w) -> l cg n c w", n=K, c=16),
                axis=AX,
            )

            # ---- transpose tokens to [c, (l,n)] (l-major columns) ----
            tokT = tokp.tile([C, LCHUNK * NTOK], bf16)
            tokT_ln = tokT.rearrange("c (l n) -> c l n", n=NTOK)
            for n in range(NTOK):
                ttp = ps_tt.tile([C, LCHUNK], bf16, tag="ttp")
                nc.tensor.transpose(ttp[:], tok_s[:, n * C : (n + 1) * C], ident_s[:])
                nc.scalar.copy(tokT_ln[:, :, n], ttp[:])

            fusedT = smallp.tile([D, LCHUNK], bf16)

            for h in range(2):
                cols = slice(h * HALF * NTOK, (h + 1) * HALF * NTOK)

                # proj = tokens @ pw + pb   -> [D, 320] (d on partitions)
                pproj = ps_sm.tile([D, HALF * NTOK], f32, tag="sm")
                nc.tensor.matmul(pproj[:], lhsT=pw_s[:], rhs=tokT[:, cols])
                projs = smallp.tile([D, HALF * NTOK], bf16, tag="projs")
                nc.scalar.add(projs[:], pproj[:], pb_s[:])

                # k / v over all tokens, q over token 0 only
                pk = ps_sm.tile([D, HALF * NTOK], f32, tag="sm")
                nc.tensor.matmul(pk[:], lhsT=kw_s[:], rhs=projs[:])
                ks = smallp.tile([D, HALF * NTOK], bf16, tag="ks")
                nc.scalar.add(ks[:], pk[:], kb_s[:])

                pv = ps_sm.tile([D, HALF * NTOK], f32, tag="sm")
                nc.tensor.matmul(pv[:], lhsT=vw_s[:], rhs=projs[:])
                vs = smallp.tile([D, HALF * NTOK], bf16, tag="vs")
                nc.scalar.add(vs[:], pv[:], vb_s[:])

                pq = ps_sm.tile([D, HALF], f32, tag="sm")
                nc.tensor.matmul(
                    pq[:],
                    lhsT=qw_s[:],
                    rhs=projs.rearrange("d (l n) -> d l n", n=NTOK)[:, :, 0],
                )
                qs = smallp.tile([D, HALF], bf16, tag="qs")
                nc.scalar.add(qs[:], pq[:], qb_s[:])

                # scores[l, n] = sum_d q[d, l] * k[d, (l,n)]
                qk = smallp.tile([D, HALF * NTOK], bf16, tag="qk")
                nc.vector.tensor_mul(
                    qk.rearrange("d (l n) -> d l n", n=NTOK),
                    ks.rearrange("d (l n) -> d l n", n=NTOK),
                    qs.unsqueeze(2).to_broadcast([D, HALF, NTOK]),
                )
                psc = ps_sm.tile([1, HALF * NTOK], f32, tag="sm")
                nc.tensor.matmul(psc[:], lhsT=ones_d[:], rhs=qk[:])

                # softmax over n (scores are O(1e-2): exp without max-shift)
                exps = smallp.tile([1, HALF * NTOK], bf16, tag="exps")
                nc.scalar.activation(exps[:], psc[:], EXP)
                den = smallp.tile([1, HALF], f32, tag="den")
                nc.vector.reduce_sum(
                    den[:], exps.rearrange("p (l n) -> p l n", n=NTOK), axis=AX
                )
                rden = smallp.tile([1, HALF], f32, tag="rden")
                nc.vector.reciprocal(rden[:], den[:])
                attn = smallp.tile([1, HALF * NTOK], bf16, tag="attn")
                nc.vector.tensor_mul(
                    attn.rearrange("p (l n) -> p l n", n=NTOK),
                    exps.rearrange("p (l n) -> p l n", n=NTOK),
                    rden.unsqueeze(2).to_broadcast([1, HALF, NTOK]),
                )

                # broadcast attn over d, weight v, reduce over n
                pab = ps_sm.tile([D, HALF * NTOK], f32, tag="sm")
                nc.tensor.matmul(pab[:], lhsT=ones_1[:], rhs=attn[:])
                av = smallp.tile([D, HALF * NTOK], bf16, tag="av")
                nc.vector.tensor_mul(av[:], vs[:], pab[:])
                nc.vector.reduce_sum(
                    fusedT[:, h * HALF : (h + 1) * HALF],
                    av.rearrange("d (l n) -> d l n", n=NTOK),
                    axis=AX,
                )

            # delta = fused @ ow + ob  -> [c, l], then transpose to [l, c]
            pdelta = ps_sm.tile([C, LCHUNK], f32, tag="sm")
            nc.tensor.matmul(pdelta[:], lhsT=ow_s[:], rhs=fusedT[:])
            deltaT = smallp.tile([C, LCHUNK], bf16, tag="deltaT")
            nc.scalar.add(deltaT[:], pdelta[:], ob_s[:])
            pdT = ps_sm.tile([LCHUNK, C], bf16, tag="sm")
            nc.tensor.transpose(pdT[:], deltaT[:], ident_s[:C, :C])

            # out = target + delta (broadcast over w), in place, then store
            nc.vector.tensor_add(
                targ[:], targ[:], pdT.unsqueeze(2).to_broadcast([LCHUNK, C, W2])
            )
            nc.sync.dma_start(
                out=y[l0 : l0 + LCHUNK].rearrange("l (c w) -> l c w", w=W2),
                in_=targ[:],
            )

    nc.compile()
    return nc


def kernel(
    target_win,
    neighbor_wins,
    proj_w,
    proj_b,
    q_w,
    q_b,
    k_w,
    k_b,
    v_w,
    v_b,
    out_w,
    out_b,
):
    global LAST_RESULTS
    import ml_dtypes

    from concourse.bass_utils import run_bass_kernel_spmd

    f = np.float32
    bf = ml_dtypes.bfloat16
    target_win = np.ascontiguousarray(np.asarray(target_win, f))
    neighbor_wins = np.ascontiguousarray(np.asarray(neighbor_wins, f))
    # Fold the window-mean (1/64) into proj_w and the 1/sqrt(D) score
    # scale into q_w/q_b (linear ops commute with these scalings).
    pw = (np.asarray(proj_w, f) / float(W2)).astype(bf)
    sc = 1.0 / math.sqrt(D)
    qw = (np.asarray(q_w, f) * sc).astype(bf)
    qb = np.asarray(q_b, f) * sc
    shared = {
        "ident": np.eye(128, dtype=bf),
        "pw": pw,
        "pb": np.asarray(proj_b, f),
        "qw": qw,
        "qb": qb,
        "kw": np.asarray(k_w, f).astype(bf),
        "kb": np.asarray(k_b, f),
        "vw": np.asarray(v_w, f).astype(bf),
        "vb": np.asarray(v_b, f),
        "ow": np.asarray(out_w, f).astype(bf),
        "ob": np.asarray(out_b, f),
    }
    in_maps = []
    for b in range(NCORES):
        in_maps.append(
            {
                "tgt": target_win[b].reshape(L, C * W2),
                "nbr": np.ascontiguousarray(
                    neighbor_wins[:, b].reshape(K, L, C * W2)
                ),
                **shared,
            }
        )

    nc = _build()
    res = run_bass_kernel_spmd(
        nc,
        in_maps,
        list(range(NCORES)),
        trace=bool(os.environ.get("KERNEL_PROFILE")),
    )
    LAST_RESULTS = res
    out = np.stack(
        [res.results[b]["y"].reshape(L, C, 8, 8) for b in range(NCORES)]
    )
    return out.astype(np.float32, copy=False)


# revision 12
# speedup vs baseline: 1.4859x; 1.0554x over previous
# Trainium2 Bass kernel for CrossScaleFreqAttention.
#
# Math (per batch b):
#   tokens[l, n, c] = mean over the 8x8 window of {target, 4 neighbors}[l, c]
#   proj = tokens @ proj_w + proj_b ; q/k/v linear ; softmax over n (5)
#   delta[l, c] = (attn-weighted v) @ out_w + out_b
#   out = target_win + delta broadcast over the window
#
# Sharding: data-parallel over B=8 -> one batch element per NeuronCore,
# weights replicated, no cross-core communication.
#
# Per-core structure (memory-bound problem: 80 MiB in + 16 MiB out per
# core at ~360 GB/s effective HBM => ~280 us roofline):
#   L=1024 in 8 chunks of 128 SBUF partitions.
#   - Neighbor window pooling on the TensorEngine: 32 accumulating
#     matmuls per chunk with a stationary bf16 identity and the f32r
#     (single-pass fp32) moving operand at N=512; the leftover w-parity
#     pair is folded with one VectorE add. This streams at 1 col/cycle
#     instead of the 1x-only VectorE reduce.
#   - Target pooling on the VectorE (its tile must stay plain f32 for
#     the exact in-place final add).
#   - Token/attention chain in bf16 (weights are bf16; every
#     contraction still accumulates in fp32 PSUM; delta is ~0.1% of the
#     output magnitude, so bf16 rounding there is ~1e-6 of the output).
#   - Final broadcast-add on the VectorE into the resident f32 target
#     tile, streamed out by DMA.

import math
import os

import numpy as np

B, L, C, W2 = 8, 1024, 64, 64
K, NTOK, D = 4, 5, 32
LCHUNK = 128
NCHUNK = L // LCHUNK
HALF = 64  # l-positions per half-chunk (320 = HALF*NTOK columns <= 512 PSUM)
NCORES = 8

LAST_RESULTS = None  # BassKernelResults of the most recent run (for test.py)


def _build():
    from contextlib import ExitStack

    import concourse.bacc as bacc
    import concourse.mybir as mybir
    import concourse.tile as tile

    f32 = mybir.dt.float32
    f32r = mybir.dt.float32r
    bf16 = mybir.dt.bfloat16
    AX = mybir.AxisListType.X
    EXP = mybir.ActivationFunctionType.Exp

    nc = bacc.Bacc(
        "TRN2",
        target_bir_lowering=False,
        debug=False,
        num_devices=NCORES,
    )

    def din(name, shape, dt=f32):
        return nc.dram_tensor(name, shape, dt, kind="ExternalInput").ap()

    tgt = din("tgt", [L, C * W2])
    nbr = din("nbr", [K, L, C * W2])
    ident = din("ident", [128, 128], bf16)
    pw = din("pw", [C, D], bf16)  # pre-scaled by 1/64 (window mean) on host
    pb = din("pb", [D])
    qw = din("qw", [D, D], bf16)  # pre-scaled by 1/sqrt(D) on host
    qb = din("qb", [D])           # pre-scaled by 1/sqrt(D) on host
    kw = din("kw", [D, D], bf16)
    kb = din("kb", [D])
    vw = din("vw", [D, D], bf16)
    vb = din("vb", [D])
    ow = din("ow", [D, C], bf16)
    ob = din("ob", [C])
    y = nc.dram_tensor("y", [L, C * W2], f32, kind="ExternalOutput").ap()

    with (
        tile.TileContext(nc) as tc,
        ExitStack() as ctx,
        nc.allow_low_precision(reason="bf16 attention path; output add stays f32"),
    ):
        const = ctx.enter_context(tc.tile_pool(name="const", bufs=1))
        bigp = ctx.enter_context(tc.tile_pool(name="big", bufs=3))
        tokp = ctx.enter_context(tc.tile_pool(name="tok", bufs=2))
        smallp = ctx.enter_context(tc.tile_pool(name="small", bufs=2))
        ps_tok = ctx.enter_context(tc.tile_pool(name="ps_tok", bufs=1, space="PSUM"))
        ps_tt = ctx.enter_context(tc.tile_pool(name="ps_tt", bufs=1, space="PSUM"))
        ps_sm = ctx.enter_context(tc.tile_pool(name="ps_sm", bufs=3, space="PSUM"))

        ident_s = const.tile([128, 128], bf16)
        nc.sync.dma_start(out=ident_s[:], in_=ident)
        pw_s = const.tile([C, D], bf16)
        nc.sync.dma_start(out=pw_s[:], in_=pw)
        qw_s = const.tile([D, D], bf16)
        nc.sync.dma_start(out=qw_s[:], in_=qw)
        kw_s = const.tile([D, D], bf16)
        nc.sync.dma_start(out=kw_s[:], in_=kw)
        vw_s = const.tile([D, D], bf16)
        nc.sync.dma_start(out=vw_s[:], in_=vw)
        ow_s = const.tile([D, C], bf16)
        nc.sync.dma_start(out=ow_s[:], in_=ow)
        pb_s = const.tile([D, 1], f32)
        nc.sync.dma_start(out=pb_s[:], in_=pb.unsqueeze(1))
        qb_s = const.tile([D, 1], f32)
        nc.sync.dma_start(out=qb_s[:], in_=qb.unsqueeze(1))
        kb_s = const.tile([D, 1], f32)
        nc.sync.dma_start(out=kb_s[:], in_=kb.unsqueeze(1))
        vb_s = const.tile([D, 1], f32)
        nc.sync.dma_start(out=vb_s[:], in_=vb.unsqueeze(1))
        ob_s = const.tile([C, 1], f32)
        nc.sync.dma_start(out=ob_s[:], in_=ob.unsqueeze(1))
        ones_d = const.tile([D, 1], bf16)
        nc.vector.memset(ones_d[:], 1.0)
        ones_1 = const.tile([1, D], bf16)
        nc.vector.memset(ones_1[:], 1.0)

        for i in range(NCHUNK):
            l0 = i * LCHUNK

            # ---- load target [128, 64, 64] f32 + neighbors [128, 4, 64, 64] f32r
            targ = bigp.tile([LCHUNK, C, W2], f32)
            nc.sync.dma_start(
                out=targ[:],
                in_=tgt[l0 : l0 + LCHUNK].rearrange("l (c w) -> l c w", w=W2),
            )
            # neighbors are cast f32 -> bf16 in the DMA engines (SWDGE):
            # HBM traffic is unchanged but the pool matmuls become pure
            # bf16 (1 col/cycle + fast weight load).
            nbig = bigp.tile([LCHUNK, K, C, W2], bf16)
            for k in range(K):
                nc.gpsimd.dma_start(
                    out=nbig[:, k],
                    in_=nbr[k, l0 : l0 + LCHUNK].rearrange("l (c w) -> l c w", w=W2),
                )

            # ---- window pooling ----
            # Neighbors on the TensorEngine. SBUF has 16-byte cachelines
            # and the moving operand pays ~4x when consecutive columns
            # hit different lines, so each matmul keeps 8 contiguous w
            # elements (= one full 16B bf16 line) innermost: 8 matmuls
            # per 16-channel group accumulate w-octets into per-w-slot
            # partial sums [128, (n, c16, w8)], and one VectorE reduce
            # folds the 8 slots. The target is pooled on the VectorE so
            # its tile stays plain f32 for the exact final add.
            tok_s = tokp.tile([LCHUNK, NTOK * C], bf16)
            ptok8 = ps_tok.tile([LCHUNK, 4, 512], f32)
            nc.vector.reduce_sum(tok_s[:, :C], targ[:], axis=AX)
            tok_n = tok_s[:, C:].rearrange("l (n cg c) -> l cg n c", n=K, cg=4)
            p8v = ptok8.rearrange("l cg (n c w) -> l cg n c w", n=K, c=16)
            for cg in range(4):
                for wo in range(8):
                    nc.tensor.matmul(
                        ptok8[:, cg],
                        lhsT=ident_s[:],
                        rhs=nbig[:, :, 16 * cg : 16 * (cg + 1), 8 * wo : 8 * (wo + 1)],
                        start=(wo == 0),
                        stop=(wo == 7),
                    )
                nc.vector.reduce_sum(tok_n[:, cg], p8v[:, cg], axis=AX)

            # ---- transpose tokens to [c, (l,n)] (l-major columns) ----
            tokT = tokp.tile([C, LCHUNK * NTOK], bf16)
            tokT_ln = tokT.rearrange("c (l n) -> c l n", n=NTOK)
            for n in range(NTOK):
                ttp = ps_tt.tile([C, LCHUNK], bf16, tag="ttp")
                nc.tensor.transpose(ttp[:], tok_s[:, n * C : (n + 1) * C], ident_s[:])
                nc.scalar.copy(tokT_ln[:, :, n], ttp[:])

            fusedT = smallp.tile([D, LCHUNK], bf16)
            exps = smallp.tile([1, LCHUNK * NTOK], bf16, tag="exps")
            projs2 = []

            for h in range(2):
                cols = slice(h * HALF * NTOK, (h + 1) * HALF * NTOK)

                # proj = tokens @ pw + pb   -> [D, 320] (d on partitions)
                pproj = ps_sm.tile([D, HALF * NTOK], f32, tag="sm")
                nc.tensor.matmul(pproj[:], lhsT=pw_s[:], rhs=tokT[:, cols])
                projs = smallp.tile([D, HALF * NTOK], bf16, tag="projs")
                nc.scalar.add(projs[:], pproj[:], pb_s[:])

                # k / v over all tokens, q over token 0 only
                pk = ps_sm.tile([D, HALF * NTOK], f32, tag="sm")
                nc.tensor.matmul(pk[:], lhsT=kw_s[:], rhs=projs[:])
                ks = smallp.tile([D, HALF * NTOK], bf16, tag="ks")
                nc.scalar.add(ks[:], pk[:], kb_s[:])

                pv = ps_sm.tile([D, HALF * NTOK], f32, tag="sm")
                nc.tensor.matmul(pv[:], lhsT=vw_s[:], rhs=projs[:])
                vs = smallp.tile([D, HALF * NTOK], bf16, tag="vs")
                nc.scalar.add(vs[:], pv[:], vb_s[:])

                pq = ps_sm.tile([D, HALF], f32, tag="sm")
                nc.tensor.matmul(
                    pq[:],
                    lhsT=qw_s[:],
                    rhs=projs.rearrange("d (l n) -> d l n", n=NTOK)[:, :, 0],
                )
                qs = smallp.tile([D, HALF], bf16, tag="qs")
                nc.scalar.add(qs[:], pq[:], qb_s[:])

                # scores[l, n] = sum_d q[d, l] * k[d, (l,n)]
                qk = smallp.tile([D, HALF * NTOK], bf16, tag="qk")
                nc.vector.tensor_mul(
                    qk.rearrange("d (l n) -> d l n", n=NTOK),
                    ks.rearrange("d (l n) -> d l n", n=NTOK),
                    qs.unsqueeze(2).to_broadcast([D, HALF, NTOK]),
                )
                psc = ps_sm.tile([1, HALF * NTOK], f32, tag="sm")
                nc.tensor.matmul(psc[:], lhsT=ones_d[:], rhs=qk[:])
                # scores are O(1e-2): exp without max-shift is exact enough
                nc.scalar.activation(exps[:, cols], psc[:], EXP)
                projs2.append(vs)

            # softmax denominator for the whole chunk at once
            den = smallp.tile([1, LCHUNK], f32, tag="den")
            nc.vector.reduce_sum(
                den[:], exps.rearrange("p (l n) -> p l n", n=NTOK), axis=AX
            )
            rden = smallp.tile([1, LCHUNK], f32, tag="rden")
            nc.vector.reciprocal(rden[:], den[:])
            attn = smallp.tile([1, LCHUNK * NTOK], bf16, tag="attn")
            nc.vector.tensor_mul(
                attn.rearrange("p (l n) -> p l n", n=NTOK),
                exps.rearrange("p (l n) -> p l n", n=NTOK),
                rden.unsqueeze(2).to_broadcast([1, LCHUNK, NTOK]),
            )

            for h in range(2):
                cols = slice(h * HALF * NTOK, (h + 1) * HALF * NTOK)
                # broadcast attn over d, weight v, reduce over n
                pab = ps_sm.tile([D, HALF * NTOK], f32, tag="sm")
                nc.tensor.matmul(pab[:], lhsT=ones_1[:], rhs=attn[:, cols])
                av = smallp.tile([D, HALF * NTOK], bf16, tag="av")
                nc.vector.tensor_mul(av[:], projs2[h][:], pab[:])
                nc.vector.reduce_sum(
                    fusedT[:, h * HALF : (h + 1) * HALF],
                    av.rearrange("d (l n) -> d l n", n=NTOK),
                    axis=AX,
                )

            # delta = fused @ ow + ob  -> [c, l], then transpose to [l, c]
            pdelta = ps_sm.tile([C, LCHUNK], f32, tag="sm")
            nc.tensor.matmul(pdelta[:], lhsT=ow_s[:], rhs=fusedT[:])
            deltaT = smallp.tile([C, LCHUNK], bf16, tag="deltaT")
            nc.scalar.add(deltaT[:], pdelta[:], ob_s[:])
            pdT = ps_sm.tile([LCHUNK, C], bf16, tag="sm")
            nc.tensor.transpose(pdT[:], deltaT[:], ident_s[:C, :C])

            # out = target + delta (broadcast over w), in place, then store
            nc.vector.tensor_add(
                targ[:], targ[:], pdT.unsqueeze(2).to_broadcast([LCHUNK, C, W2])
            )
            nc.sync.dma_start(
                out=y[l0 : l0 + LCHUNK].rearrange("l (c w) -> l c w", w=W2),
                in_=targ[:],
            )

    nc.compile()
    return nc


def kernel(
    target_win,
    neighbor_wins,
    proj_w,
    proj_b,
    q_w,
    q_b,
    k_w,
    k_b,
    v_w,
    v_b,
    out_w,
    out_b,
):
    global LAST_RESULTS
    import ml_dtypes

    from concourse.bass_utils import run_bass_kernel_spmd

    f = np.float32
    bf = ml_dtypes.bfloat16
    target_win = np.ascontiguousarray(np.asarray(target_win, f))
    neighbor_wins = np.ascontiguousarray(np.asarray(neighbor_wins, f))
    # Fold the window-mean (1/64) into proj_w and the 1/sqrt(D) score
    # scale into q_w/q_b (linear ops commute with these scalings).
    pw = (np.asarray(proj_w, f) / float(W2)).astype(bf)
    sc = 1.0 / math.sqrt(D)
    qw = (np.asarray(q_w, f) * sc).astype(bf)
    qb = np.asarray(q_b, f) * sc
    shared = {
        "ident": np.eye(128, dtype=bf),
        "pw": pw,
        "pb": np.asarray(proj_b, f),
        "qw": qw,
        "qb": qb,
        "kw": np.asarray(k_w, f).astype(bf),
        "kb": np.asarray(k_b, f),
        "vw": np.asarray(v_w, f).astype(bf),
        "vb": np.asarray(v_b, f),
        "ow": np.asarray(out_w, f).astype(bf),
        "ob": np.asarray(out_b, f),
    }
    in_maps = []
    for b in range(NCORES):
        in_maps.append(
            {
                "tgt": target_win[b].reshape(L, C * W2),
                "nbr": np.ascontiguousarray(
                    neighbor_wins[:, b].reshape(K, L, C * W2)
                ),
                **shared,
            }
        )

    nc = _build()
    res = run_bass_kernel_spmd(
        nc,
        in_maps,
        list(range(NCORES)),
        trace=bool(os.environ.get("KERNEL_PROFILE")),
    )
    LAST_RESULTS = res
    out = np.stack(
        [res.results[b]["y"].reshape(L, C, 8, 8) for b in range(NCORES)]
    )
    return out.astype(np.float32, copy=False)
